# revision 13
# baseline (speedup 1.0000x reference)
"""Trainium2 Bass kernel for the fused soft-logic-gate layer.

Reference computation:
    pa = softmax(wa, axis=1); pb = softmax(wb, axis=1); pt = softmax(wt, axis=0)
    A = pa @ x; B = pb @ x
    out = sum_g pt[g,:,None] * gate_g(A, B)        (16 soft logic gates)

Every gate is affine in {1, A, B, A*B}, so the 16-gate table collapses to
    out = c0 + cA*A + cB*B + cAB*(A*B)
with four per-row coefficient vectors derived from pt, and factoring
    out = (A + u) * (cAB*B + cA) + w,   u = cB/cAB,  w = c0 - cA*u
leaves three elementwise passes per tile, split across ACT and DVE.

The weights are inference-time constants, so all of the softmax /
coefficient algebra is folded on the host (float64).  The matmuls run in
fp8e4 DoubleRow mode (full K=256 reduction per instruction): pa/pb rows
are rescaled to the fp8 range (row max → 224) and the inverse scales are
folded into the epilogue constants, x is quantized to fp8e4, and the
output is written as bf16 and upcast on the host.  Measured end-to-end
relative error ≈ 8e-3 against the float64 reference (tolerance 2e-2).

Pipeline: x streams in six chunks (512/512/1024/1024/512/512 columns —
small edge chunks shorten the pipeline fill and drain) on the sync HWDGE
ring while weights and the m=0 output groups ride the scalar ring, so
the two rings hide each other's completion-receipt gaps.  The epilogue
is spread across ACT (s = cAB*B + cA from PSUM), DVE (p = (A+u)*s from
PSUM), and GPSIMD (out = p + w, SBUF only) so no single engine paces the
matmul stream.

Sharding: batch axis of x split evenly across 8 NeuronCores (data
parallel), weights replicated.
"""

import os
import sys

for _p in ("/opt/trn_rl_repo",):
    if _p not in sys.path and os.path.isdir(_p):
        sys.path.insert(0, _p)

import numpy as np
import ml_dtypes

SIZE = 256
PREV = 256
BATCH = 32768
N_CORES = 8
BSH = BATCH // N_CORES  # per-core batch shard
P = 128

# column widths of the x chunks (sum = BSH); small edge chunks so the
# pipeline starts earlier and drains faster
WIDTHS = [512, 512, 1024, 1024, 512, 512]
OFFS = [0, 512, 1024, 2048, 3072, 3584]
# output DMA groups: (start col, width, last chunk index in the group)
OGROUPS = [(0, 1024, 1), (1024, 1024, 2), (2048, 1024, 3),
           (3072, 512, 4), (3584, 512, 5)]

_CACHE = {}


def _build_bass():
    import concourse.bacc as bacc
    import concourse.tile as tile
    import concourse.mybir as mybir

    f32 = mybir.dt.float32
    f8 = mybir.dt.float8e4
    bf16 = mybir.dt.bfloat16
    Act = mybir.ActivationFunctionType
    Alu = mybir.AluOpType
    DR = mybir.MatmulPerfMode.DoubleRow

    nc = bacc.Bacc(trn_type="TRN2", target_bir_lowering=False, debug=False,
                   num_devices=N_CORES)

    xs_d = nc.dram_tensor("xs", [PREV, BSH], f8, kind="ExternalInput").ap()
    wb_d = nc.dram_tensor("wblob", [P, 1024], f8, kind="ExternalInput").ap()
    cf_d = nc.dram_tensor("cf", [P, 8], f32, kind="ExternalInput").ap()
    out_d = nc.dram_tensor("out", [SIZE, BSH], bf16, kind="ExternalOutput").ap()

    # [p, k, b] view for single-DMA transfers
    xs_v = xs_d.rearrange("(k p) b -> p k b", p=P)

    with tile.TileContext(nc) as tc:
        with tc.tile_pool(name="consts", bufs=1) as consts, \
             tc.tile_pool(name="xp", bufs=len(WIDTHS)) as xp:

            # x chunks on the sync ring — first (small) chunk first so the
            # matmul pipeline starts as early as possible
            xtiles = []
            for ci, (w, off) in enumerate(zip(WIDTHS, OFFS)):
                xt = xp.tile([P, 2, w], f8, tag=f"x{w}", name=f"x{ci}")
                nc.sync.dma_start(out=xt[:], in_=xs_v[:, :, off:off + w])
                xtiles.append(xt)

            # constants on the scalar ring, concurrent with x0; weights
            # first (they gate the matmuls), cf afterwards
            w_sb = consts.tile([P, 1024], f8)
            nc.scalar.dma_start(out=w_sb[:], in_=wb_d[:])
            cf_sb = consts.tile([P, 8], f32)
            nc.scalar.dma_start(out=cf_sb[:], in_=cf_d[:])

            # tiny early ACT op forces the table load off the critical path
            dummy = consts.tile([1, 1], f32)
            nc.scalar.activation(out=dummy[:], in_=cf_sb[0:1, 0:1],
                                 func=Act.Identity)

            # DoubleRow lhsT views: [128, 2(k), 128(m)] fp8, layout
            # wblob[p, a/b*512 + k*256 + m]
            w_ap = w_sb[:].rearrange("p (w k m) -> p w k m", w=2, k=2)

            with tc.tile_pool(name="ep", bufs=3) as ep, \
                 tc.tile_pool(name="og", bufs=3) as og, \
                 tc.tile_pool(name="mm_ps", bufs=2, space="PSUM") as mm_ps:
                obig = {}
                pending = []  # deferred ACT-side +w ops: (gi, m, dst, src, bias)

                def emit_dma(gi, m):
                    # m=0 outputs on the scalar ring, m=1 on sync
                    go, gw, gl = OGROUPS[gi]
                    eng = nc.scalar if m == 0 else nc.sync
                    ot = obig.pop((gi, m))
                    eng.dma_start(out=out_d[m * P:(m + 1) * P, go:go + gw],
                                  in_=ot[:])

                def flush_pending():
                    for (gi, m, last, dst, src, bias) in pending:
                        nc.scalar.activation(out=dst, in_=src,
                                             func=Act.Identity, bias=bias)
                        if last:
                            emit_dma(gi, m)
                    pending.clear()

                for ci, (w, off) in enumerate(zip(WIDTHS, OFFS)):
                    xk = xtiles[ci]
                    gi = next(i for i, (go, gw, gl) in enumerate(OGROUPS)
                              if go <= off < go + gw)
                    go, gw, gl = OGROUPS[gi]
                    for m in range(2):
                        if (gi, m) not in obig:
                            obig[(gi, m)] = og.tile([P, gw], bf16, tag="o",
                                                    name=f"o{gi}{m}")
                        a_ps = mm_ps.tile([P, 1024], f32, tag="A", name=f"A{ci}{m}")
                        b_ps = mm_ps.tile([P, 1024], f32, tag="B", name=f"B{ci}{m}")
                        for ps_t, wsel in ((a_ps, 0), (b_ps, 1)):
                            lhsT = w_ap[:, wsel, :, m * P:(m + 1) * P]
                            for so in range(0, w, 512):
                                sl = slice(so, min(so + 512, w))
                                nc.tensor.matmul(ps_t[:, sl], lhsT,
                                                 xk[:, :, sl],
                                                 start=True, stop=True,
                                                 perf_mode=DR)
                        # out = (A' + u') * (sc*B' + sb) + w
                        s_sb = ep.tile([P, w], f32, tag=f"s{w}", name=f"s{ci}{m}")
                        nc.scalar.activation(out=s_sb[:], in_=b_ps[:, :w],
                                             func=Act.Identity,
                                             scale=cf_sb[:, 4 + m:5 + m],
                                             bias=cf_sb[:, 2 + m:3 + m])
                        # emit the previous iteration's deferred ACT +w
                        # after this iteration's s op, so it never
                        # head-of-line-blocks s in the strict-FIFO ACT queue
                        if m == 1:
                            flush_pending()
                        p_sb = ep.tile([P, w], f32, tag=f"p{w}", name=f"p{ci}{m}")
                        nc.vector.scalar_tensor_tensor(out=p_sb[:], in0=a_ps[:, :w],
                                                       scalar=cf_sb[:, m:m + 1],
                                                       in1=s_sb[:],
                                                       op0=Alu.add, op1=Alu.mult)
                        lo = off - go
                        # +w split by width: narrow chunks ride DVE's
                        # cheap 2x single-src mode, wide ones go to ACT
                        # (deferred one iteration so they never block an
                        # s op in the strict-FIFO ACT queue); GPSIMD
                        # tensor ops are ~10x slower than DVE (measured),
                        # so it gets none
                        if w < 1024:
                            nc.vector.tensor_scalar_add(
                                obig[(gi, m)][:, lo:lo + w], p_sb[:],
                                cf_sb[:, 6 + m:7 + m])
                            if ci == gl:
                                emit_dma(gi, m)
                        else:
                            pending.append((gi, m, ci == gl,
                                            obig[(gi, m)][:, lo:lo + w],
                                            p_sb[:], cf_sb[:, 6 + m:7 + m]))
                flush_pending()

    nc.compile()
    return nc


def _get_nc():
    if "nc" not in _CACHE:
        _CACHE["nc"] = _build_bass()
    return _CACHE["nc"]


def _softmax64(a: np.ndarray, axis: int) -> np.ndarray:
    a = a.astype(np.float64)
    e = np.exp(a - a.max(axis=axis, keepdims=True))
    return e / e.sum(axis=axis, keepdims=True)


def _fold_weights(wa: np.ndarray, wb: np.ndarray, wt: np.ndarray):
    """Fold softmax + gate-table algebra into device constants (float64)."""
    pa = _softmax64(wa, 1)
    pb = _softmax64(wb, 1)
    pt = _softmax64(wt, 0)
    # gate-table coefficients of {1, A, B, A*B}
    S = np.zeros((16, 4), dtype=np.float64)
    S[8:16, 0] = 1.0
    for g in (2, 3, 6, 7):
        S[g, 1] += 1.0
    for g in (8, 9, 12, 13):
        S[g, 1] -= 1.0
    for g in (4, 5, 6, 7):
        S[g, 2] += 1.0
    for g in (8, 9, 10, 11):
        S[g, 2] -= 1.0
    for g, v in {1: 1, 2: -1, 4: -1, 6: -2, 7: -1, 8: 1, 9: 2, 11: 1, 13: 1, 14: -1}.items():
        S[g, 3] = v
    c = pt.T @ S  # [256, 4] = c0, cA, cB, cAB per row
    c0, cA, cB, cAB = c[:, 0], c[:, 1], c[:, 2], c[:, 3]
    u = cB / cAB
    w = c0 - cA * u

    # fp8 row scaling: row max -> 224 (TRN e4m3 max normal is 240)
    f8 = ml_dtypes.float8_e4m3
    ra = 224.0 / pa.max(axis=1)
    rb = 224.0 / pb.max(axis=1)
    pa8 = (pa * ra[:, None]).astype(np.float32).astype(f8)
    pb8 = (pb * rb[:, None]).astype(np.float32).astype(f8)

    # wblob[p, ab*512 + k*256 + m] = p?8[m, k*128+p]
    wblob = np.empty((P, 1024), dtype=f8)
    for ab, p8 in ((0, pa8), (1, pb8)):
        t = p8.T.reshape(2, P, SIZE).transpose(1, 0, 2).reshape(P, 512)
        wblob[:, ab * 512:(ab + 1) * 512] = t

    # A = A'/ra, B = B'/rb; fold the scales into the epilogue constants:
    #   s = (cAB/(ra*rb))*B' + cA/ra ; out = (A' + u*ra)*s + w
    cf = np.zeros((P, 8), dtype=np.float32)
    for m in range(2):
        blk = slice(m * P, (m + 1) * P)
        cf[:, 0 + m] = (u * ra)[blk]
        cf[:, 2 + m] = (cA / ra)[blk]
        cf[:, 4 + m] = (cAB / (ra * rb))[blk]
        cf[:, 6 + m] = w[blk]
    return wblob, cf


def _run(x, wa, wb, wt, trace=False, **spmd_kwargs):
    from concourse import bass_utils

    nc = _get_nc()
    x8 = np.ascontiguousarray(
        np.asarray(x, dtype=np.float32).astype(ml_dtypes.float8_e4m3))
    wblob, cf = _fold_weights(np.asarray(wa), np.asarray(wb), np.asarray(wt))

    in_maps = []
    for c in range(N_CORES):
        in_maps.append({
            "xs": np.ascontiguousarray(x8[:, c * BSH:(c + 1) * BSH]),
            "wblob": wblob, "cf": cf,
        })
    res = bass_utils.run_bass_kernel_spmd(nc, in_maps, core_ids=list(range(N_CORES)),
                                          trace=trace, **spmd_kwargs)
    out = np.concatenate([res.results[c]["out"] for c in range(N_CORES)],
                         axis=1).astype(np.float32)
    return out, res


def kernel(x, wa, wb, wt):
    out, _ = _run(x, wa, wb, wt, trace=False)
    return out


# revision 18
# speedup vs baseline: 1.0550x; 1.0550x over previous
"""Trainium2 Bass kernel for the fused soft-logic-gate layer.

Reference computation:
    pa = softmax(wa, axis=1); pb = softmax(wb, axis=1); pt = softmax(wt, axis=0)
    A = pa @ x; B = pb @ x
    out = sum_g pt[g,:,None] * gate_g(A, B)        (16 soft logic gates)

Every gate is affine in {1, A, B, A*B}, so the 16-gate table collapses to
    out = c0 + cA*A + cB*B + cAB*(A*B)
with four per-row coefficient vectors derived from pt, and factoring
    out = (A + u) * (cAB*B + cA) + w,   u = cB/cAB,  w = c0 - cA*u
leaves three elementwise passes per tile, split across ACT and DVE.

The weights are inference-time constants, so all of the softmax /
coefficient algebra is folded on the host (float64).  The matmuls run in
fp8e4 DoubleRow mode (full K=256 reduction per instruction): pa/pb rows
are rescaled to the fp8 range (row max → 224) and the inverse scales are
folded into the epilogue constants, x is quantized to fp8e4, and the
output is written as bf16 and upcast on the host.  Measured end-to-end
relative error ≈ 8e-3 against the float64 reference (tolerance 2e-2).

Pipeline: x streams in six chunks (512/512/1024/1024/512/512 columns —
small edge chunks shorten the pipeline fill and drain) on the sync HWDGE
ring while weights and the m=0 output groups ride the scalar ring, so
the two rings hide each other's completion-receipt gaps.  The epilogue
is spread across ACT (s = cAB*B + cA from PSUM; +w for m=1) and DVE
(p = (A+u)*s from PSUM; +w for m=0) so neither engine paces the matmul
stream.

Sharding: batch axis of x split evenly across 8 NeuronCores (data
parallel), weights replicated.
"""

import os
import sys

for _p in ("/opt/trn_rl_repo",):
    if _p not in sys.path and os.path.isdir(_p):
        sys.path.insert(0, _p)

import numpy as np
import ml_dtypes

SIZE = 256
PREV = 256
BATCH = 32768
N_CORES = 8
BSH = BATCH // N_CORES  # per-core batch shard
P = 128

# column widths of the x chunks (sum = BSH); small edge chunks so the
# pipeline starts earlier and drains faster
WIDTHS = [512, 512, 1024, 1024, 512, 512]
OFFS = [0, 512, 1024, 2048, 3072, 3584]
# output DMA groups: (start col, width, last chunk index in the group)
OGROUPS = [(0, 1024, 1), (1024, 1024, 2), (2048, 1024, 3), (3072, 1024, 5)]

_CACHE = {}


def _build_bass():
    import concourse.bacc as bacc
    import concourse.tile as tile
    import concourse.mybir as mybir

    f32 = mybir.dt.float32
    f8 = mybir.dt.float8e4
    bf16 = mybir.dt.bfloat16
    Act = mybir.ActivationFunctionType
    Alu = mybir.AluOpType
    DR = mybir.MatmulPerfMode.DoubleRow

    nc = bacc.Bacc(trn_type="TRN2", target_bir_lowering=False, debug=False,
                   num_devices=N_CORES)

    xs_d = nc.dram_tensor("xs", [PREV, BSH], f8, kind="ExternalInput").ap()
    wb_d = nc.dram_tensor("wblob", [P, 1024], f8, kind="ExternalInput").ap()
    cf_d = nc.dram_tensor("cf", [P, 8], f32, kind="ExternalInput").ap()
    out_d = nc.dram_tensor("out", [SIZE, BSH], bf16, kind="ExternalOutput").ap()

    # [p, k, b] view for single-DMA transfers
    xs_v = xs_d.rearrange("(k p) b -> p k b", p=P)

    with tile.TileContext(nc) as tc:
        with tc.tile_pool(name="consts", bufs=1) as consts, \
             tc.tile_pool(name="xp", bufs=len(WIDTHS)) as xp:

            # x chunks on the sync ring — first (small) chunk first so the
            # matmul pipeline starts as early as possible
            xtiles = []
            for ci, (w, off) in enumerate(zip(WIDTHS, OFFS)):
                xt = xp.tile([P, 2, w], f8, tag=f"x{w}", name=f"x{ci}")
                nc.sync.dma_start(out=xt[:], in_=xs_v[:, :, off:off + w])
                xtiles.append(xt)

            # constants on the scalar ring, concurrent with x0; weights
            # first (they gate the matmuls), cf afterwards
            w_sb = consts.tile([P, 1024], f8)
            nc.scalar.dma_start(out=w_sb[:], in_=wb_d[:])
            cf_sb = consts.tile([P, 8], f32)
            nc.scalar.dma_start(out=cf_sb[:], in_=cf_d[:])

            # tiny early ACT op forces the table load off the critical path
            dummy = consts.tile([1, 1], f32)
            nc.scalar.activation(out=dummy[:], in_=cf_sb[0:1, 0:1],
                                 func=Act.Identity)

            # DoubleRow lhsT views: [128, 2(k), 128(m)] fp8, layout
            # wblob[p, a/b*512 + k*256 + m]
            w_ap = w_sb[:].rearrange("p (w k m) -> p w k m", w=2, k=2)

            with tc.tile_pool(name="ep", bufs=3) as ep, \
                 tc.tile_pool(name="og", bufs=3) as og, \
                 tc.tile_pool(name="mm_ps", bufs=2, space="PSUM") as mm_ps:
                obig = {}

                def emit_dma(gi, m):
                    # m=0 outputs on the scalar ring, m=1 on sync; the
                    # final group is split so the last receipt is short
                    go, gw, gl = OGROUPS[gi]
                    eng = nc.scalar if m == 0 else nc.sync
                    ot = obig.pop((gi, m))
                    if gi == len(OGROUPS) - 1 and m == 1:
                        hw = gw // 2
                        for h in range(2):
                            eng.dma_start(
                                out=out_d[m * P:(m + 1) * P,
                                          go + h * hw:go + (h + 1) * hw],
                                in_=ot[:, h * hw:(h + 1) * hw])
                    else:
                        eng.dma_start(out=out_d[m * P:(m + 1) * P, go:go + gw],
                                      in_=ot[:])

                for ci, (w, off) in enumerate(zip(WIDTHS, OFFS)):
                    xk = xtiles[ci]
                    gi = next(i for i, (go, gw, gl) in enumerate(OGROUPS)
                              if go <= off < go + gw)
                    go, gw, gl = OGROUPS[gi]
                    for m in range(2):
                        if (gi, m) not in obig:
                            obig[(gi, m)] = og.tile([P, gw], bf16, tag="o",
                                                    name=f"o{gi}{m}")
                        a_ps = mm_ps.tile([P, 1024], f32, tag="A", name=f"A{ci}{m}")
                        b_ps = mm_ps.tile([P, 1024], f32, tag="B", name=f"B{ci}{m}")
                        for ps_t, wsel in ((a_ps, 0), (b_ps, 1)):
                            lhsT = w_ap[:, wsel, :, m * P:(m + 1) * P]
                            for so in range(0, w, 512):
                                sl = slice(so, min(so + 512, w))
                                nc.tensor.matmul(ps_t[:, sl], lhsT,
                                                 xk[:, :, sl],
                                                 start=True, stop=True,
                                                 perf_mode=DR)
                        # out = (A' + u') * (sc*B' + sb) + w
                        s_sb = ep.tile([P, w], f32, tag=f"s{w}", name=f"s{ci}{m}")
                        nc.scalar.activation(out=s_sb[:], in_=b_ps[:, :w],
                                             func=Act.Identity,
                                             scale=cf_sb[:, 4 + m:5 + m],
                                             bias=cf_sb[:, 2 + m:3 + m])
                        p_sb = ep.tile([P, w], f32, tag=f"p{w}", name=f"p{ci}{m}")
                        nc.vector.scalar_tensor_tensor(out=p_sb[:], in0=a_ps[:, :w],
                                                       scalar=cf_sb[:, m:m + 1],
                                                       in1=s_sb[:],
                                                       op0=Alu.add, op1=Alu.mult)
                        lo = off - go
                        # +w split across DVE (m=0, cheap 2x single-src
                        # mode) and ACT (m=1) so neither engine paces the
                        # stream; GPSIMD tensor ops are ~10x slower than
                        # DVE (measured), so it gets none
                        if m == 0:
                            nc.vector.tensor_scalar_add(
                                obig[(gi, m)][:, lo:lo + w], p_sb[:],
                                cf_sb[:, 6 + m:7 + m])
                        else:
                            nc.scalar.activation(
                                out=obig[(gi, m)][:, lo:lo + w], in_=p_sb[:],
                                func=Act.Identity,
                                bias=cf_sb[:, 6 + m:7 + m])
                        if ci == gl:
                            emit_dma(gi, m)

    nc.compile()
    return nc


def _get_nc():
    if "nc" not in _CACHE:
        _CACHE["nc"] = _build_bass()
    return _CACHE["nc"]


def _softmax64(a: np.ndarray, axis: int) -> np.ndarray:
    a = a.astype(np.float64)
    e = np.exp(a - a.max(axis=axis, keepdims=True))
    return e / e.sum(axis=axis, keepdims=True)


def _fold_weights(wa: np.ndarray, wb: np.ndarray, wt: np.ndarray):
    """Fold softmax + gate-table algebra into device constants (float64)."""
    pa = _softmax64(wa, 1)
    pb = _softmax64(wb, 1)
    pt = _softmax64(wt, 0)
    # gate-table coefficients of {1, A, B, A*B}
    S = np.zeros((16, 4), dtype=np.float64)
    S[8:16, 0] = 1.0
    for g in (2, 3, 6, 7):
        S[g, 1] += 1.0
    for g in (8, 9, 12, 13):
        S[g, 1] -= 1.0
    for g in (4, 5, 6, 7):
        S[g, 2] += 1.0
    for g in (8, 9, 10, 11):
        S[g, 2] -= 1.0
    for g, v in {1: 1, 2: -1, 4: -1, 6: -2, 7: -1, 8: 1, 9: 2, 11: 1, 13: 1, 14: -1}.items():
        S[g, 3] = v
    c = pt.T @ S  # [256, 4] = c0, cA, cB, cAB per row
    c0, cA, cB, cAB = c[:, 0], c[:, 1], c[:, 2], c[:, 3]
    u = cB / cAB
    w = c0 - cA * u

    # fp8 row scaling: row max -> 224 (TRN e4m3 max normal is 240)
    f8 = ml_dtypes.float8_e4m3
    ra = 224.0 / pa.max(axis=1)
    rb = 224.0 / pb.max(axis=1)
    pa8 = (pa * ra[:, None]).astype(np.float32).astype(f8)
    pb8 = (pb * rb[:, None]).astype(np.float32).astype(f8)

    # wblob[p, ab*512 + k*256 + m] = p?8[m, k*128+p]
    wblob = np.empty((P, 1024), dtype=f8)
    for ab, p8 in ((0, pa8), (1, pb8)):
        t = p8.T.reshape(2, P, SIZE).transpose(1, 0, 2).reshape(P, 512)
        wblob[:, ab * 512:(ab + 1) * 512] = t

    # A = A'/ra, B = B'/rb; fold the scales into the epilogue constants:
    #   s = (cAB/(ra*rb))*B' + cA/ra ; out = (A' + u*ra)*s + w
    cf = np.zeros((P, 8), dtype=np.float32)
    for m in range(2):
        blk = slice(m * P, (m + 1) * P)
        cf[:, 0 + m] = (u * ra)[blk]
        cf[:, 2 + m] = (cA / ra)[blk]
        cf[:, 4 + m] = (cAB / (ra * rb))[blk]
        cf[:, 6 + m] = w[blk]
    return wblob, cf


def _run(x, wa, wb, wt, trace=False, **spmd_kwargs):
    from concourse import bass_utils

    nc = _get_nc()
    x8 = np.ascontiguousarray(
        np.asarray(x, dtype=np.float32).astype(ml_dtypes.float8_e4m3))
    wblob, cf = _fold_weights(np.asarray(wa), np.asarray(wb), np.asarray(wt))

    in_maps = []
    for c in range(N_CORES):
        in_maps.append({
            "xs": np.ascontiguousarray(x8[:, c * BSH:(c + 1) * BSH]),
            "wblob": wblob, "cf": cf,
        })
    res = bass_utils.run_bass_kernel_spmd(nc, in_maps, core_ids=list(range(N_CORES)),
                                          trace=trace, **spmd_kwargs)
    out = np.concatenate([res.results[c]["out"] for c in range(N_CORES)],
                         axis=1).astype(np.float32)
    return out, res


def kernel(x, wa, wb, wt):
    out, _ = _run(x, wa, wb, wt, trace=False)
    return out


# revision 21
# speedup vs baseline: 1.0615x; 1.0062x over previous
"""Trainium2 Bass kernel for the fused soft-logic-gate layer.

Reference computation:
    pa = softmax(wa, axis=1); pb = softmax(wb, axis=1); pt = softmax(wt, axis=0)
    A = pa @ x; B = pb @ x
    out = sum_g pt[g,:,None] * gate_g(A, B)        (16 soft logic gates)

Every gate is affine in {1, A, B, A*B}, so the 16-gate table collapses to
    out = c0 + cA*A + cB*B + cAB*(A*B)
with four per-row coefficient vectors derived from pt, and factoring
    out = (A + u) * (cAB*B + cA) + w,   u = cB/cAB,  w = c0 - cA*u
leaves three elementwise passes per tile, split across ACT and DVE.

The weights are inference-time constants, so all of the softmax /
coefficient algebra is folded on the host (float64).  The matmuls run in
fp8e4 DoubleRow mode (full K=256 reduction per instruction): pa/pb rows
are rescaled to the fp8 range (row max → 224) and the inverse scales are
folded into the epilogue constants, x is quantized to fp8e4, and the
output is written as bf16 and upcast on the host.  Measured end-to-end
relative error ≈ 8e-3 against the float64 reference (tolerance 2e-2).

Pipeline: x streams in six chunks (512/512/1024/1024/512/512 columns —
small edge chunks shorten the pipeline fill and drain) on the sync HWDGE
ring while weights and the m=0 output groups ride the scalar ring, so
the two rings hide each other's completion-receipt gaps.  The epilogue
is spread across ACT (s = cAB*B + cA from PSUM; +w for m=1) and DVE
(p = (A+u)*s from PSUM; +w for m=0) so neither engine paces the matmul
stream.

Sharding: batch axis of x split evenly across 8 NeuronCores (data
parallel), weights replicated.
"""

import os
import sys

for _p in ("/opt/trn_rl_repo",):
    if _p not in sys.path and os.path.isdir(_p):
        sys.path.insert(0, _p)

import numpy as np
import ml_dtypes

SIZE = 256
PREV = 256
BATCH = 32768
N_CORES = 8
BSH = BATCH // N_CORES  # per-core batch shard
P = 128

# column widths of the x chunks (sum = BSH); small edge chunks so the
# pipeline starts earlier and drains faster
WIDTHS = [512, 512, 1024, 1024, 512, 512]
OFFS = [0, 512, 1024, 2048, 3072, 3584]
# output DMA groups: (start col, width, last chunk index in the group)
OGROUPS = [(0, 1024, 1), (1024, 1024, 2), (2048, 1024, 3), (3072, 1024, 5)]

_CACHE = {}


def _build_bass():
    import concourse.bacc as bacc
    import concourse.tile as tile
    import concourse.mybir as mybir

    f32 = mybir.dt.float32
    f8 = mybir.dt.float8e4
    bf16 = mybir.dt.bfloat16
    Act = mybir.ActivationFunctionType
    Alu = mybir.AluOpType
    DR = mybir.MatmulPerfMode.DoubleRow

    nc = bacc.Bacc(trn_type="TRN2", target_bir_lowering=False, debug=False,
                   num_devices=N_CORES)

    xs_d = nc.dram_tensor("xs", [PREV, BSH], f8, kind="ExternalInput").ap()
    wb_d = nc.dram_tensor("wblob", [P, 1024], f8, kind="ExternalInput").ap()
    cf_d = nc.dram_tensor("cf", [P, 8], f32, kind="ExternalInput").ap()
    out_d = nc.dram_tensor("out", [SIZE, BSH], bf16, kind="ExternalOutput").ap()

    # [p, k, b] view for single-DMA transfers
    xs_v = xs_d.rearrange("(k p) b -> p k b", p=P)

    with tile.TileContext(nc) as tc:
        with tc.tile_pool(name="consts", bufs=1) as consts, \
             tc.tile_pool(name="xp", bufs=len(WIDTHS)) as xp:

            # x chunks on the sync ring — first (small) chunk first so the
            # matmul pipeline starts as early as possible
            xtiles = []
            for ci, (w, off) in enumerate(zip(WIDTHS, OFFS)):
                xt = xp.tile([P, 2, w], f8, tag=f"x{w}", name=f"x{ci}")
                nc.sync.dma_start(out=xt[:], in_=xs_v[:, :, off:off + w])
                xtiles.append(xt)

            # constants on the scalar ring, concurrent with x0; weights
            # first (they gate the matmuls), cf afterwards
            w_sb = consts.tile([P, 1024], f8)
            nc.scalar.dma_start(out=w_sb[:], in_=wb_d[:])
            cf_sb = consts.tile([P, 8], f32)
            nc.scalar.dma_start(out=cf_sb[:], in_=cf_d[:])

            # tiny early ACT op forces the table load off the critical path
            dummy = consts.tile([1, 1], f32)
            nc.scalar.activation(out=dummy[:], in_=cf_sb[0:1, 0:1],
                                 func=Act.Identity)

            # DoubleRow lhsT views: [128, 2(k), 128(m)] fp8, layout
            # wblob[p, a/b*512 + k*256 + m]
            w_ap = w_sb[:].rearrange("p (w k m) -> p w k m", w=2, k=2)

            # zero tiles for PE warm-up matmuls (memset so the race
            # detector sees them initialized)
            warm_w = consts.tile([P, 2, P], f8)
            nc.vector.memset(warm_w[:], 0)
            warm_x = consts.tile([P, 2, 512], f8)
            nc.vector.memset(warm_x[:], 0)

            with tc.tile_pool(name="ep", bufs=3) as ep, \
                 tc.tile_pool(name="og", bufs=3) as og, \
                 tc.tile_pool(name="mm_ps", bufs=2, space="PSUM") as mm_ps:
                # dummy matmuls during the x0 DMA wait: the PE's HAM clock
                # gate needs ~3.4us of sustained activity to release the
                # 1.2GHz cold throttle, so burn the wait warming it up.
                # They write one A-pool buffer; the WAW dep resolves before
                # the first real matmul needs that buffer again.
                a_warm = mm_ps.tile([P, 1024], f32, tag="A", name="warm")
                for i in range(4):
                    nc.tensor.matmul(a_warm[:, 0:512], warm_w[:], warm_x[:],
                                     start=True, stop=True, perf_mode=DR)

                obig = {}

                def emit_dma(gi, m):
                    # m=0 outputs on the scalar ring, m=1 on sync; the
                    # final group is split so the last receipt is short
                    go, gw, gl = OGROUPS[gi]
                    eng = nc.scalar if m == 0 else nc.sync
                    ot = obig.pop((gi, m))
                    if gi == len(OGROUPS) - 1 and m == 1:
                        hw = gw // 2
                        for h in range(2):
                            eng.dma_start(
                                out=out_d[m * P:(m + 1) * P,
                                          go + h * hw:go + (h + 1) * hw],
                                in_=ot[:, h * hw:(h + 1) * hw])
                    else:
                        eng.dma_start(out=out_d[m * P:(m + 1) * P, go:go + gw],
                                      in_=ot[:])

                for ci, (w, off) in enumerate(zip(WIDTHS, OFFS)):
                    xk = xtiles[ci]
                    gi = next(i for i, (go, gw, gl) in enumerate(OGROUPS)
                              if go <= off < go + gw)
                    go, gw, gl = OGROUPS[gi]
                    for m in range(2):
                        if (gi, m) not in obig:
                            obig[(gi, m)] = og.tile([P, gw], bf16, tag="o",
                                                    name=f"o{gi}{m}")
                        a_ps = mm_ps.tile([P, 1024], f32, tag="A", name=f"A{ci}{m}")
                        b_ps = mm_ps.tile([P, 1024], f32, tag="B", name=f"B{ci}{m}")
                        for ps_t, wsel in ((a_ps, 0), (b_ps, 1)):
                            lhsT = w_ap[:, wsel, :, m * P:(m + 1) * P]
                            for so in range(0, w, 512):
                                sl = slice(so, min(so + 512, w))
                                nc.tensor.matmul(ps_t[:, sl], lhsT,
                                                 xk[:, :, sl],
                                                 start=True, stop=True,
                                                 perf_mode=DR)
                        # out = (A' + u') * (sc*B' + sb) + w
                        s_sb = ep.tile([P, w], f32, tag=f"s{w}", name=f"s{ci}{m}")
                        nc.scalar.activation(out=s_sb[:], in_=b_ps[:, :w],
                                             func=Act.Identity,
                                             scale=cf_sb[:, 4 + m:5 + m],
                                             bias=cf_sb[:, 2 + m:3 + m])
                        p_sb = ep.tile([P, w], f32, tag=f"p{w}", name=f"p{ci}{m}")
                        nc.vector.scalar_tensor_tensor(out=p_sb[:], in0=a_ps[:, :w],
                                                       scalar=cf_sb[:, m:m + 1],
                                                       in1=s_sb[:],
                                                       op0=Alu.add, op1=Alu.mult)
                        lo = off - go
                        # +w split so both engines carry ~4 units of it
                        # and the last chunk drains through DVE (the ACT
                        # queue is the fuller one at the end); GPSIMD
                        # tensor ops are ~10x slower than DVE (measured),
                        # so it gets none
                        on_dve = (m == 0 and ci != 3) or (m == 1 and ci in (0, 5))
                        if on_dve:
                            nc.vector.tensor_scalar_add(
                                obig[(gi, m)][:, lo:lo + w], p_sb[:],
                                cf_sb[:, 6 + m:7 + m])
                        else:
                            nc.scalar.activation(
                                out=obig[(gi, m)][:, lo:lo + w], in_=p_sb[:],
                                func=Act.Identity,
                                bias=cf_sb[:, 6 + m:7 + m])
                        if ci == gl:
                            emit_dma(gi, m)

    nc.compile()
    return nc


def _get_nc():
    if "nc" not in _CACHE:
        _CACHE["nc"] = _build_bass()
    return _CACHE["nc"]


def _softmax64(a: np.ndarray, axis: int) -> np.ndarray:
    a = a.astype(np.float64)
    e = np.exp(a - a.max(axis=axis, keepdims=True))
    return e / e.sum(axis=axis, keepdims=True)


def _fold_weights(wa: np.ndarray, wb: np.ndarray, wt: np.ndarray):
    """Fold softmax + gate-table algebra into device constants (float64)."""
    pa = _softmax64(wa, 1)
    pb = _softmax64(wb, 1)
    pt = _softmax64(wt, 0)
    # gate-table coefficients of {1, A, B, A*B}
    S = np.zeros((16, 4), dtype=np.float64)
    S[8:16, 0] = 1.0
    for g in (2, 3, 6, 7):
        S[g, 1] += 1.0
    for g in (8, 9, 12, 13):
        S[g, 1] -= 1.0
    for g in (4, 5, 6, 7):
        S[g, 2] += 1.0
    for g in (8, 9, 10, 11):
        S[g, 2] -= 1.0
    for g, v in {1: 1, 2: -1, 4: -1, 6: -2, 7: -1, 8: 1, 9: 2, 11: 1, 13: 1, 14: -1}.items():
        S[g, 3] = v
    c = pt.T @ S  # [256, 4] = c0, cA, cB, cAB per row
    c0, cA, cB, cAB = c[:, 0], c[:, 1], c[:, 2], c[:, 3]
    u = cB / cAB
    w = c0 - cA * u

    # fp8 row scaling: row max -> 224 (TRN e4m3 max normal is 240)
    f8 = ml_dtypes.float8_e4m3
    ra = 224.0 / pa.max(axis=1)
    rb = 224.0 / pb.max(axis=1)
    pa8 = (pa * ra[:, None]).astype(np.float32).astype(f8)
    pb8 = (pb * rb[:, None]).astype(np.float32).astype(f8)

    # wblob[p, ab*512 + k*256 + m] = p?8[m, k*128+p]
    wblob = np.empty((P, 1024), dtype=f8)
    for ab, p8 in ((0, pa8), (1, pb8)):
        t = p8.T.reshape(2, P, SIZE).transpose(1, 0, 2).reshape(P, 512)
        wblob[:, ab * 512:(ab + 1) * 512] = t

    # A = A'/ra, B = B'/rb; fold the scales into the epilogue constants:
    #   s = (cAB/(ra*rb))*B' + cA/ra ; out = (A' + u*ra)*s + w
    cf = np.zeros((P, 8), dtype=np.float32)
    for m in range(2):
        blk = slice(m * P, (m + 1) * P)
        cf[:, 0 + m] = (u * ra)[blk]
        cf[:, 2 + m] = (cA / ra)[blk]
        cf[:, 4 + m] = (cAB / (ra * rb))[blk]
        cf[:, 6 + m] = w[blk]
    return wblob, cf


def _run(x, wa, wb, wt, trace=False, **spmd_kwargs):
    from concourse import bass_utils

    nc = _get_nc()
    x8 = np.ascontiguousarray(
        np.asarray(x, dtype=np.float32).astype(ml_dtypes.float8_e4m3))
    wblob, cf = _fold_weights(np.asarray(wa), np.asarray(wb), np.asarray(wt))

    in_maps = []
    for c in range(N_CORES):
        in_maps.append({
            "xs": np.ascontiguousarray(x8[:, c * BSH:(c + 1) * BSH]),
            "wblob": wblob, "cf": cf,
        })
    res = bass_utils.run_bass_kernel_spmd(nc, in_maps, core_ids=list(range(N_CORES)),
                                          trace=trace, **spmd_kwargs)
    out = np.concatenate([res.results[c]["out"] for c in range(N_CORES)],
                         axis=1).astype(np.float32)
    return out, res


def kernel(x, wa, wb, wt):
    out, _ = _run(x, wa, wb, wt, trace=False)
    return out
